# revision 31
# baseline (speedup 1.0000x reference)
"""Distributed Trainium2 Bass kernel for nn_AttentionBlock_76115410419715.

Math (B=4, S=2048, D=64, H=12; softmax over the QUERY axis):
    qp = q@Wq+bq, kp = q@Wk+bk, vp = q@Wv+bv          (per-head blocks of 64)
    s[b,h,q,k] = qp . kp / 8
    attn = exp(s) / colsum_q(exp(s))                   [softmax over q]
    ctx[b,q,h,:] = sum_k attn[q,k] vp[k,:]
    out = ctx @ Wo + bo

Sharding: (batch, head-half) across 8 cores — core c handles batch c//2 and
heads [6*(c%2), 6*(c%2)+6). Each core computes a partial out^T [64, 2048];
a grouped psum over core pairs {2b, 2b+1} (dispatched on-device right after
the bass NEFF) produces the full output for batch b (each core adds bo/2).

Per-core flash-style pipeline, all in SBUF (scores never hit HBM):
  - projections with bias folded in via an appended ones-row (contraction 65)
  - heads processed in PAIRS: head 2i on PE rows/cols 0-63, head 2i+1 on
    64-127, so score matmuls (K=64, row-tiled) and ctx matmuls (M=64,
    col-tiled) of the two heads execute concurrently in the PE array
  - per k-chunk of 128: scores^T on PE (bf16); exp + column-sum either on
    ACT (activation Exp, accum_out) or on DVE (2 custom fused ops:
    deg-4 poly of e^(s/64) then three squarings, with accum) — the split
    keeps both engines busy
  - V/Z normalisation on DVE, ctx^T accumulated in PSUM, out-proj per head
    accumulated into SBUF f32.
"""

import sys

if "/opt/trn_rl_repo" not in sys.path:
    sys.path.insert(0, "/opt/trn_rl_repo")

import numpy as np

import concourse.bass as bass
import concourse.tile as tile
from concourse import mybir

B, S, D, H = 4, 2048, 64, 12
N_CORES = 8
HPC = 6          # heads per core
HB = HPC * D     # 384, per-core head-block width
KC = S // 128    # 16 k-chunks
F32 = mybir.dt.float32
BF16 = mybir.dt.bfloat16
REPLICA_GROUPS = [[0, 1], [2, 3], [4, 5], [6, 7]]

# i16-bits exp: exp(s/8) = 2^y, y = s*0.125*log2(e); i16 = y*128 + HACK_B
# then bitcast to bf16 (8-bit exponent, 7-bit mantissa -> (1+frac)*2^floor).
C_SHIFT = 0.043  # centers the (1+f)/2^f sawtooth (rel err to +-3%)
HACK_A = 0.125 * np.log2(np.e) * 128.0
HACK_B = (127.0 - C_SHIFT) * 128.0
# kc indices (staggered per sub) whose exp runs as the DVE hack instead of
# ACT exp -- the ACT/DVE load-balance knob (4 of 16 kc per sub = 24 chunks)
def _use_hack(kc, sub):
    return (kc + 2 * sub) % 4 == 3




def _fix_drain_waits(nc):
    """This walrus build rejects instructions carrying >1 sem wait; move
    extras onto same-engine NOPs inserted immediately before (same engine
    stream => identical blocking semantics)."""
    eng = {
        mybir.EngineType.SP: nc.sync,
        mybir.EngineType.Pool: nc.gpsimd,
        mybir.EngineType.DVE: nc.vector,
        mybir.EngineType.Activation: nc.scalar,
        mybir.EngineType.PE: nc.tensor,
    }
    for bb in nc.main_func.blocks:
        fixes = []
        for idx, ins in enumerate(bb.instructions):
            si = ins.sync_info
            if (
                si is not None
                and si.on_wait is not None
                and len(si.on_wait) > 1
                and ins.engine in eng
            ):
                fixes.append((idx, ins))
        for idx, ins in reversed(fixes):
            si = ins.sync_info
            waits = list(si.on_wait)
            si.on_wait[:] = waits[-1:]
            nops = []
            for w in waits[:-1]:
                bi = eng[ins.engine].nop(nofuse=True, hint="split_wait")
                nop_ins = bi.ins
                for bb2 in nc.main_func.blocks:
                    if nop_ins in bb2.instructions:
                        bb2.instructions.remove(nop_ins)
                        break
                nsi = nop_ins.sync_info
                if nsi is None:
                    nop_ins.sync_info = type(si)(on_wait=[w], on_update=[])
                else:
                    nsi.on_wait[:] = [w]
                nops.append(nop_ins)
            for j, nop_ins in enumerate(nops):
                bb.instructions.insert(idx + j, nop_ins)


def _build():
    nc = bass.Bass(num_devices=N_CORES)

    qt_ext = nc.declare_dram_parameter("qt", [D, S], F32, isOutput=False)
    wq_ext = nc.declare_dram_parameter("wq", [D, HB], F32, isOutput=False)
    bq_ext = nc.declare_dram_parameter("bq", [HB], F32, isOutput=False)
    wk_ext = nc.declare_dram_parameter("wk", [D, HB], F32, isOutput=False)
    bk_ext = nc.declare_dram_parameter("bk", [HB], F32, isOutput=False)
    wv_ext = nc.declare_dram_parameter("wv", [D, HB], F32, isOutput=False)
    bv_ext = nc.declare_dram_parameter("bv", [HB], F32, isOutput=False)
    wo_ext = nc.declare_dram_parameter("wo", [HB, D], F32, isOutput=False)
    bo_ext = nc.declare_dram_parameter("bo", [D], F32, isOutput=False)
    out_ext = nc.declare_dram_parameter("out", [D, S], F32, isOutput=True)

    with tile.TileContext(nc) as tc:
        with (
            tc.tile_pool(name="const", bufs=1) as const,
            tc.tile_pool(name="ld", bufs=2) as ld,
            tc.tile_pool(name="qk", bufs=1) as qk,
            tc.tile_pool(name="vp", bufs=1) as vpool,
            tc.tile_pool(name="ep", bufs=4) as ep,
            tc.tile_pool(name="small", bufs=4) as small,
            tc.tile_pool(name="cs", bufs=2) as cs,
            tc.tile_pool(name="scp0", bufs=1, space="PSUM") as scp0,
            tc.tile_pool(name="scp1", bufs=1, space="PSUM") as scp1,
            tc.tile_pool(name="ctxp", bufs=1, space="PSUM") as ctxp,
        ):
            scp = (scp0, scp1)

            # ---- load + prep constants -------------------------------------
            qte = const.tile([D + 1, S], BF16, tag="qte")
            qt_f32 = ld.tile([D, S], F32, tag="ldq")
            nc.gpsimd.dma_start(out=qt_f32[:], in_=qt_ext[:])
            nc.vector.tensor_copy(qte[0:D, :], qt_f32[:])
            nc.vector.memset(qte[D : D + 1, :], 1.0)

            def load_we(w_ext, b_ext, tag):
                we = const.tile([D + 1, HB], BF16, tag=tag)
                w_f32 = ld.tile([D, HB], F32, tag="ldw")
                nc.gpsimd.dma_start(out=w_f32[:], in_=w_ext[:])
                nc.vector.tensor_copy(we[0:D, :], w_f32[:])
                b_f32 = ld.tile([1, HB], F32, tag="ldb")
                nc.gpsimd.dma_start(
                    out=b_f32[:], in_=b_ext.rearrange("(a b) -> a b", a=1)
                )
                nc.vector.tensor_copy(we[D : D + 1, :], b_f32[:])
                return we

            wq_e = load_we(wq_ext, bq_ext, "wq")
            wk_e = load_we(wk_ext, bk_ext, "wk")
            wv_e = load_we(wv_ext, bv_ext, "wv")

            # Wo duplicated on both partition halves (lhsT for the col-tiled
            # out-proj of the odd head must sit at base partition 64)
            wo_pair = const.tile([128, HPC, D], BF16, tag="wo")
            wo_f32 = ld.tile([D, HB], F32, tag="ldw")
            nc.gpsimd.dma_start(
                out=wo_f32.rearrange("a (h b) -> a h b", h=HPC),
                in_=wo_ext.rearrange("(h a) b -> a h b", h=HPC),
            )
            nc.vector.tensor_copy(
                wo_pair[0:D, :, :], wo_f32.rearrange("a (h b) -> a h b", h=HPC)
            )
            nc.scalar.copy(
                wo_pair[D : 2 * D, :, :], wo_f32.rearrange("a (h b) -> a h b", h=HPC)
            )

            bo_t = const.tile([D, 1], F32, tag="bo")
            nc.gpsimd.dma_start(
                out=bo_t[:], in_=bo_ext.rearrange("(a b) -> a b", b=1)
            )
            nc.vector.tensor_scalar_mul(bo_t[:], bo_t[:], 0.5)

            out_acc = const.tile([D, S], F32, tag="out_acc")

            # write-only scratch for the Pool z-sum (accum is the output)
            zscr = const.tile([128, S], BF16, tag="zscr")

            # ---- projections ----------------------------------------------
            # V natural layout: v_sb[sc][s(128), HB]
            v_sb = []
            for sc in range(KC):
                v_ps = scp[sc % 2].tile([128, HB], F32, tag=f"sc{sc % 2}")
                nc.tensor.matmul(
                    v_ps[:], qte[:, sc * 128 : (sc + 1) * 128], wv_e[:],
                    start=True, stop=True,
                )
                vt = vpool.tile([128, HB], BF16, tag=f"v{sc}")
                if sc % 2 == 0:
                    nc.vector.tensor_copy(vt[:], v_ps[:])
                else:
                    nc.scalar.copy(vt[:], v_ps[:])
                v_sb.append(vt)

            # Q^T / K^T: per head-pair tiles [128 (2 heads x 64 dout), S]
            qt_sb, kt_sb = [], []
            for p in range(HPC // 2):
                for (we, dst_list, tg) in ((wq_e, qt_sb, "q"), (wk_e, kt_sb, "k")):
                    t = qk.tile([128, S], BF16, tag=f"{tg}{p}")
                    for qc in range(4):
                        pps = scp[qc % 2].tile([128, 512], F32, tag=f"sc{qc % 2}")
                        nc.tensor.matmul(
                            pps[:],
                            we[:, p * 128 : (p + 1) * 128],
                            qte[:, qc * 512 : (qc + 1) * 512],
                            start=True, stop=True,
                        )
                        if qc % 2 == 0:
                            nc.vector.tensor_copy(
                                t[:, qc * 512 : (qc + 1) * 512], pps[:]
                            )
                        else:
                            nc.scalar.copy(t[:, qc * 512 : (qc + 1) * 512], pps[:])
                    dst_list.append(t)

            # ---- attention, one head-pair at a time -----------------------
            for p in range(HPC // 2):
                # pre-zeroed ctx accumulator; all ctx matmuls run with
                # start=False so accumulate-or-overwrite both land on the
                # matmul result over zeros regardless of the bank's stale
                # per-element has_written state (the two heads interleave
                # in one bank, so a start=True whole-bank clear would race)
                ctx_ps = ctxp.tile([128, S], F32, tag="ctx")
                nc.vector.memset(ctx_ps[:], 0.0)

                def emit_vn(kc, sub, zp_t, z_add, p=p):
                    # normalisation chain, one iteration ahead of its ctx
                    # matmuls: Pool add -> DVE recip -> Pool mul (Pool is
                    # otherwise idle; keeps big DVE ops off this path)
                    h = 2 * p + sub
                    if z_add:
                        z_t = small.tile([128, 1], F32, tag=f"z{sub}")
                        nc.vector.tensor_add(z_t[:], zp_t[:, 0:1], zp_t[:, 1:2])
                    else:
                        z_t = zp_t
                    zr_t = small.tile([128, 1], F32, tag=f"zr{sub}")
                    nc.vector.reciprocal(zr_t[:], z_t[:, 0:1])
                    vn_t = small.tile([128, D], BF16, tag=f"vn{sub}")
                    nc.vector.tensor_scalar_mul(
                        vn_t[:], v_sb[kc][:, h * D : (h + 1) * D], zr_t[:]
                    )
                    return vn_t

                def emit_ctx(sub, e_t, vn_t, ctx_ps=ctx_ps):
                    for qc in range(4):
                        nc.tensor.matmul(
                            ctx_ps[sub * D : (sub + 1) * D,
                                   qc * 512 : (qc + 1) * 512],
                            vn_t[:],
                            e_t[:, qc * 512 : (qc + 1) * 512],
                            start=False, stop=False,
                            skip_group_check=True,
                        )

                # pend[sub]: deque of (kc, e_t, zp_t, z_add) / vn results
                pend_z = {0: [], 1: []}   # awaiting vn chain (depth 1)
                pend_c = {0: [], 1: []}   # awaiting ctx matmuls (depth 1)
                for kc in range(KC):
                    for sub in (0, 1):
                        po = D * sub
                        use_hack = _use_hack(kc, sub)
                        # 1) ctx matmuls for kc-2 (vn from last iteration --
                        #    guaranteed ready, so the in-order PE queue
                        #    never stalls here)
                        if len(pend_c[sub]) > 1:
                            ce_t, cvn_t = pend_c[sub].pop(0)
                            emit_ctx(sub, ce_t, cvn_t)
                        # 2) scores + exp/hack for kc; the hack halves go
                        #    FIRST in the DVE queue so the score PSUM bank
                        #    frees as early as possible (PE's next scores
                        #    wait on it)
                        e_t = ep.tile([128, S], BF16, tag=f"e{sub}")
                        zp_t = small.tile([128, 2], F32, tag=f"zp{sub}")
                        for half in (0, 1):
                            s_t = scp[sub].tile([128, 1024], F32, tag=f"sc{sub}")
                            for qq in (0, 1):
                                j = half * 2 + qq
                                nc.tensor.matmul(
                                    s_t[:, qq * 512 : (qq + 1) * 512],
                                    kt_sb[p][po : po + D, kc * 128 : (kc + 1) * 128],
                                    qt_sb[p][po : po + D, j * 512 : (j + 1) * 512],
                                    start=True, stop=True,
                                )
                            esl = e_t[:, half * 1024 : (half + 1) * 1024]
                            if use_hack:
                                # exp(s/8) = 2^(s/8*log2e) via integer bits:
                                # i16 = s*(16*log2e) + (127 - C_SHIFT)*128,
                                # bitcast to bf16 (same exponent layout).
                                # Sawtooth rel err ~+-3% cancels in the ctx
                                # k-sum (verified 2.8e-3 end-to-end).
                                nc.vector.tensor_scalar(
                                    esl.bitcast(mybir.dt.int16), s_t[:],
                                    HACK_A, HACK_B,
                                    op0=mybir.AluOpType.mult,
                                    op1=mybir.AluOpType.add,
                                )
                            else:
                                # exp + z-sum in one ACT pass (accumulator
                                # read costs only ~280ns vs a 1x-mode DVE
                                # reduce at ~1.4us)
                                nc.scalar.activation(
                                    esl, s_t[:],
                                    mybir.ActivationFunctionType.Exp,
                                    scale=0.125,
                                    accum_out=zp_t[:, half : half + 1],
                                )
                        # 3) vn chain for kc-1 (z ready by now) -- after the
                        #    hack halves, before the bulky z-pass
                        if pend_z[sub]:
                            zkc, ze_t, zzp, zadd = pend_z[sub].pop(0)
                            vn_t = emit_vn(zkc, sub, zzp, zadd)
                            pend_c[sub].append((ze_t, vn_t))
                        # 4) z for hack chunks: tensor_scalar accum pass
                        #    (1x mode), last in the DVE queue -- only the
                        #    NEXT iteration's vn chain needs it
                        if use_hack:
                            nc.vector.tensor_scalar(
                                zscr[:], e_t[:], 1.0, None,
                                op0=mybir.AluOpType.mult,
                                op1=mybir.AluOpType.add,
                                accum_out=zp_t[:, 0:1],
                            )
                        pend_z[sub].append((kc, e_t, zp_t, not use_hack))
                # drain the pipeline
                for sub in (0, 1):
                    while pend_z[sub] or pend_c[sub]:
                        if pend_z[sub]:
                            zkc, ze_t, zzp, zadd = pend_z[sub].pop(0)
                            vn_t = emit_vn(zkc, sub, zzp, zadd)
                            pend_c[sub].append((ze_t, vn_t))
                        ce_t, cvn_t = pend_c[sub].pop(0)
                        emit_ctx(sub, ce_t, cvn_t)

                ctx_sb = cs.tile([128, S], BF16, tag="ctx_sb")
                nc.vector.tensor_copy(ctx_sb[:], ctx_ps[:])

                for sub in (0, 1):
                    h = 2 * p + sub
                    po = D * sub
                    for qc in range(4):
                        o_ps = scp[sub].tile([D, 512], F32, tag=f"sc{sub}")
                        nc.tensor.matmul(
                            o_ps[:],
                            wo_pair[po : po + D, h, :],
                            ctx_sb[po : po + D, qc * 512 : (qc + 1) * 512],
                            start=True, stop=True,
                        )
                        sl = slice(qc * 512, (qc + 1) * 512)
                        if p == 0 and sub == 0:
                            nc.vector.tensor_copy(out_acc[:, sl], o_ps[:])
                        else:
                            nc.vector.tensor_add(
                                out_acc[:, sl], out_acc[:, sl], o_ps[:]
                            )

            # ---- epilogue: half-bias + store partial (pair-reduced by a
            # grouped psum fused into the same jitted dispatch) --------------
            nc.vector.tensor_scalar_add(out_acc[:], out_acc[:], bo_t[:])
            nc.gpsimd.dma_start(out=out_ext[:], in_=out_acc[:])

    _fix_drain_waits(nc)
    return nc


def shard_inputs(q, Wq, bq, Wk, bk, Wv, bv, Wo, bo):
    in_maps = []
    for c in range(N_CORES):
        b, j = c // 2, c % 2
        hs = slice(j * HB, (j + 1) * HB)
        in_maps.append(
            {
                "qt": np.ascontiguousarray(q[b].T, dtype=np.float32),
                "wq": np.ascontiguousarray(Wq[:, hs], dtype=np.float32),
                "bq": np.ascontiguousarray(bq[hs], dtype=np.float32),
                "wk": np.ascontiguousarray(Wk[:, hs], dtype=np.float32),
                "bk": np.ascontiguousarray(bk[hs], dtype=np.float32),
                "wv": np.ascontiguousarray(Wv[:, hs], dtype=np.float32),
                "bv": np.ascontiguousarray(bv[hs], dtype=np.float32),
                "wo": np.ascontiguousarray(Wo[hs, :], dtype=np.float32),
                "bo": np.ascontiguousarray(bo, dtype=np.float32),
            }
        )
    return in_maps


_CACHE = {}


def get_nc():
    if "nc" not in _CACHE:
        _CACHE["nc"] = _build()
    return _CACHE["nc"]


def run_spmd(nc, in_maps):
    """run_bass_via_pjrt with a grouped psum dispatched on-device right
    after the bass NEFF (the NEFF-embedded collective_compute hangs under
    this runtime, so the pair-reduction runs as an XLA collective; the
    bass_exec jit must contain only the custom call, so the psum is its
    own dispatch on device-resident outputs)."""
    import jax
    from jax.sharding import Mesh, PartitionSpec
    from jax.experimental.shard_map import shard_map
    from concourse import bass2jax

    bass2jax.install_neuronx_cc_hook()

    partition_name = nc.partition_id_tensor.name if nc.partition_id_tensor else None
    in_names, out_names, out_avals, zero_outs = [], [], [], []
    for alloc in nc.m.functions[0].allocations:
        if not isinstance(alloc, mybir.MemoryLocationSet):
            continue
        name = alloc.memorylocations[0].name
        if alloc.kind == "ExternalInput":
            if name != partition_name:
                in_names.append(name)
        elif alloc.kind == "ExternalOutput":
            out_names.append(name)
            shape = tuple(alloc.tensor_shape)
            dtype = mybir.dt.np(alloc.dtype)
            out_avals.append(jax.core.ShapedArray(shape, dtype))
            zero_outs.append(np.zeros(shape, dtype))
    n_params = len(in_names)
    n_outs = len(out_avals)
    in_names = in_names + out_names
    if partition_name is not None:
        in_names.append(partition_name)
    donate = tuple(range(n_params, n_params + n_outs))

    def _body(*args):
        operands = list(args)
        if partition_name is not None:
            operands.append(bass2jax.partition_id_tensor())
        outs = bass2jax._bass_exec_p.bind(
            *operands,
            out_avals=tuple(out_avals),
            in_names=tuple(in_names),
            out_names=tuple(out_names),
            lowering_input_output_aliases=(),
            sim_require_finite=True,
            sim_require_nnan=True,
            nc=nc,
        )
        return tuple(outs)

    devices = jax.devices()[:N_CORES]
    mesh = Mesh(np.asarray(devices), ("core",))
    sharded = jax.jit(
        shard_map(
            _body,
            mesh=mesh,
            in_specs=(PartitionSpec("core"),) * (n_params + n_outs),
            out_specs=(PartitionSpec("core"),) * n_outs,
            check_rep=False,
        ),
        donate_argnums=donate,
        keep_unused=True,
    )
    per_core = [[np.asarray(m[name]) for name in in_names[:n_params]] for m in in_maps]
    concat_in = [
        np.concatenate([per_core[c][i] for c in range(N_CORES)], axis=0)
        for i in range(n_params)
    ]
    concat_zeros = [
        np.zeros((N_CORES * z.shape[0], *z.shape[1:]), z.dtype) for z in zero_outs
    ]
    out_arrs = sharded(*concat_in, *concat_zeros)

    # pair-reduce on device: separate dispatch (the bass_exec jit must
    # contain only the custom call, per neuronx_cc_hook's checks)
    def _reduce(*outs):
        return tuple(
            jax.lax.psum(o, "core", axis_index_groups=REPLICA_GROUPS) for o in outs
        )

    reducer = jax.jit(
        shard_map(
            _reduce,
            mesh=mesh,
            in_specs=(PartitionSpec("core"),) * n_outs,
            out_specs=(PartitionSpec("core"),) * n_outs,
            check_rep=False,
        )
    )
    out_arrs = reducer(*out_arrs)
    return [
        {
            name: np.asarray(out_arrs[i]).reshape(N_CORES, *out_avals[i].shape)[c]
            for i, name in enumerate(out_names)
        }
        for c in range(N_CORES)
    ]


def kernel(q, Wq, bq, Wk, bk, Wv, bv, Wo, bo):
    nc = get_nc()
    in_maps = shard_inputs(q, Wq, bq, Wk, bk, Wv, bv, Wo, bo)
    results = run_spmd(nc, in_maps)
    out = np.stack([results[2 * b]["out"].T for b in range(B)], axis=0)
    return out.astype(np.float32)



# revision 34
# speedup vs baseline: 1.2209x; 1.2209x over previous
"""Distributed Trainium2 Bass kernel for nn_AttentionBlock_76115410419715.

Math (B=4, S=2048, D=64, H=12; softmax over the QUERY axis):
    qp = q@Wq+bq, kp = q@Wk+bk, vp = q@Wv+bv          (per-head blocks of 64)
    s[b,h,q,k] = qp . kp / 8
    attn = exp(s) / colsum_q(exp(s))                   [softmax over q]
    ctx[b,q,h,:] = sum_k attn[q,k] vp[k,:]
    out = ctx @ Wo + bo

Sharding: (batch, head-half) across 8 cores — core c handles batch c//2 and
heads [6*(c%2), 6*(c%2)+6). Each core computes a partial out^T [64, 2048];
a grouped psum over core pairs {2b, 2b+1} (dispatched on-device right after
the bass NEFF) produces the full output for batch b (each core adds bo/2).

Per-core flash-style pipeline, all in SBUF (scores never hit HBM):
  - projections with bias folded in via an appended ones-row (contraction 65)
  - heads processed in PAIRS: head 2i on PE rows/cols 0-63, head 2i+1 on
    64-127, so score matmuls (K=64, row-tiled) and ctx matmuls (M=64,
    col-tiled) of the two heads execute concurrently in the PE array
  - per k-chunk of 128: scores^T on PE (bf16); exp + column-sum either on
    ACT (activation Exp, accum_out) or on DVE (2 custom fused ops:
    deg-4 poly of e^(s/64) then three squarings, with accum) — the split
    keeps both engines busy
  - V/Z normalisation on DVE, ctx^T accumulated in PSUM, out-proj per head
    accumulated into SBUF f32.
"""

import sys

if "/opt/trn_rl_repo" not in sys.path:
    sys.path.insert(0, "/opt/trn_rl_repo")

import numpy as np

import concourse.bass as bass
import concourse.tile as tile
from concourse import mybir

B, S, D, H = 4, 2048, 64, 12
N_CORES = 8
HPC = 6          # heads per core
HB = HPC * D     # 384, per-core head-block width
KC = S // 128    # 16 k-chunks
F32 = mybir.dt.float32
BF16 = mybir.dt.bfloat16
REPLICA_GROUPS = [[0, 1], [2, 3], [4, 5], [6, 7]]

# i16-bits exp: exp(s/8) = 2^y, y = s*0.125*log2(e); i16 = y*128 + HACK_B
# then bitcast to bf16 (8-bit exponent, 7-bit mantissa -> (1+frac)*2^floor).
C_SHIFT = 0.043  # centers the (1+f)/2^f sawtooth (rel err to +-3%)
HACK_A = 0.125 * np.log2(np.e) * 128.0
HACK_B = (127.0 - C_SHIFT) * 128.0
# kc indices (staggered per sub) whose exp runs as the DVE hack instead of
# ACT exp -- the ACT/DVE load-balance knob (4 of 16 kc per sub = 24 chunks)
def _use_hack(kc, sub):
    return (kc + 2 * sub) % 4 == 3




def _fix_drain_waits(nc):
    """This walrus build rejects instructions carrying >1 sem wait; move
    extras onto same-engine NOPs inserted immediately before (same engine
    stream => identical blocking semantics)."""
    eng = {
        mybir.EngineType.SP: nc.sync,
        mybir.EngineType.Pool: nc.gpsimd,
        mybir.EngineType.DVE: nc.vector,
        mybir.EngineType.Activation: nc.scalar,
        mybir.EngineType.PE: nc.tensor,
    }
    for bb in nc.main_func.blocks:
        fixes = []
        for idx, ins in enumerate(bb.instructions):
            si = ins.sync_info
            if (
                si is not None
                and si.on_wait is not None
                and len(si.on_wait) > 1
                and ins.engine in eng
            ):
                fixes.append((idx, ins))
        for idx, ins in reversed(fixes):
            si = ins.sync_info
            waits = list(si.on_wait)
            si.on_wait[:] = waits[-1:]
            nops = []
            for w in waits[:-1]:
                bi = eng[ins.engine].nop(nofuse=True, hint="split_wait")
                nop_ins = bi.ins
                for bb2 in nc.main_func.blocks:
                    if nop_ins in bb2.instructions:
                        bb2.instructions.remove(nop_ins)
                        break
                nsi = nop_ins.sync_info
                if nsi is None:
                    nop_ins.sync_info = type(si)(on_wait=[w], on_update=[])
                else:
                    nsi.on_wait[:] = [w]
                nops.append(nop_ins)
            for j, nop_ins in enumerate(nops):
                bb.instructions.insert(idx + j, nop_ins)


def _build():
    nc = bass.Bass(num_devices=N_CORES)

    qt_ext = nc.declare_dram_parameter("qt", [D, S], F32, isOutput=False)
    wq_ext = nc.declare_dram_parameter("wq", [D, HB], F32, isOutput=False)
    bq_ext = nc.declare_dram_parameter("bq", [HB], F32, isOutput=False)
    wk_ext = nc.declare_dram_parameter("wk", [D, HB], F32, isOutput=False)
    bk_ext = nc.declare_dram_parameter("bk", [HB], F32, isOutput=False)
    wv_ext = nc.declare_dram_parameter("wv", [D, HB], F32, isOutput=False)
    bv_ext = nc.declare_dram_parameter("bv", [HB], F32, isOutput=False)
    wo_ext = nc.declare_dram_parameter("wo", [HB, D], F32, isOutput=False)
    bo_ext = nc.declare_dram_parameter("bo", [D], F32, isOutput=False)
    out_ext = nc.declare_dram_parameter("out", [D, S], F32, isOutput=True)

    with tile.TileContext(nc) as tc:
        with (
            tc.tile_pool(name="const", bufs=1) as const,
            tc.tile_pool(name="ld", bufs=2) as ld,
            tc.tile_pool(name="qk", bufs=1) as qk,
            tc.tile_pool(name="vp", bufs=1) as vpool,
            tc.tile_pool(name="ep", bufs=6) as ep,
            tc.tile_pool(name="small", bufs=8) as small,
            tc.tile_pool(name="cs", bufs=2) as cs,
            tc.tile_pool(name="scp0", bufs=1, space="PSUM") as scp0,
            tc.tile_pool(name="scp1", bufs=1, space="PSUM") as scp1,
            tc.tile_pool(name="ctxp", bufs=1, space="PSUM") as ctxp,
        ):
            scp = (scp0, scp1)

            # ---- load + prep constants -------------------------------------
            qte = const.tile([D + 1, S], BF16, tag="qte")
            qt_f32 = ld.tile([D, S], F32, tag="ldq")
            nc.gpsimd.dma_start(out=qt_f32[:], in_=qt_ext[:])
            nc.vector.tensor_copy(qte[0:D, :], qt_f32[:])
            nc.vector.memset(qte[D : D + 1, :], 1.0)

            def load_we(w_ext, b_ext, tag):
                we = const.tile([D + 1, HB], BF16, tag=tag)
                w_f32 = ld.tile([D, HB], F32, tag="ldw")
                nc.gpsimd.dma_start(out=w_f32[:], in_=w_ext[:])
                nc.vector.tensor_copy(we[0:D, :], w_f32[:])
                b_f32 = ld.tile([1, HB], F32, tag="ldb")
                nc.gpsimd.dma_start(
                    out=b_f32[:], in_=b_ext.rearrange("(a b) -> a b", a=1)
                )
                nc.vector.tensor_copy(we[D : D + 1, :], b_f32[:])
                return we

            wq_e = load_we(wq_ext, bq_ext, "wq")
            wk_e = load_we(wk_ext, bk_ext, "wk")
            wv_e = load_we(wv_ext, bv_ext, "wv")

            # Wo duplicated on both partition halves (lhsT for the col-tiled
            # out-proj of the odd head must sit at base partition 64)
            wo_pair = const.tile([128, HPC, D], BF16, tag="wo")
            wo_f32 = ld.tile([D, HB], F32, tag="ldw")
            nc.gpsimd.dma_start(
                out=wo_f32.rearrange("a (h b) -> a h b", h=HPC),
                in_=wo_ext.rearrange("(h a) b -> a h b", h=HPC),
            )
            nc.vector.tensor_copy(
                wo_pair[0:D, :, :], wo_f32.rearrange("a (h b) -> a h b", h=HPC)
            )
            nc.scalar.copy(
                wo_pair[D : 2 * D, :, :], wo_f32.rearrange("a (h b) -> a h b", h=HPC)
            )

            bo_t = const.tile([D, 1], F32, tag="bo")
            nc.gpsimd.dma_start(
                out=bo_t[:], in_=bo_ext.rearrange("(a b) -> a b", b=1)
            )
            nc.vector.tensor_scalar_mul(bo_t[:], bo_t[:], 0.5)

            out_acc = const.tile([D, S], F32, tag="out_acc")

            # write-only scratch for the Pool z-sum (accum is the output)
            zscr = const.tile([128, S], BF16, tag="zscr")

            # ---- projections ----------------------------------------------
            # V natural layout: v_sb[sc][s(128), HB]
            v_sb = []
            for sc in range(KC):
                v_ps = scp[sc % 2].tile([128, HB], F32, tag=f"sc{sc % 2}")
                nc.tensor.matmul(
                    v_ps[:], qte[:, sc * 128 : (sc + 1) * 128], wv_e[:],
                    start=True, stop=True,
                )
                vt = vpool.tile([128, HB], BF16, tag=f"v{sc}")
                if sc % 2 == 0:
                    nc.vector.tensor_copy(vt[:], v_ps[:])
                else:
                    nc.scalar.copy(vt[:], v_ps[:])
                v_sb.append(vt)

            # Q^T / K^T: per head-pair tiles [128 (2 heads x 64 dout), S]
            qt_sb, kt_sb = [], []
            for p in range(HPC // 2):
                for (we, dst_list, tg) in ((wq_e, qt_sb, "q"), (wk_e, kt_sb, "k")):
                    t = qk.tile([128, S], BF16, tag=f"{tg}{p}")
                    for qc in range(4):
                        pps = scp[qc % 2].tile([128, 512], F32, tag=f"sc{qc % 2}")
                        nc.tensor.matmul(
                            pps[:],
                            we[:, p * 128 : (p + 1) * 128],
                            qte[:, qc * 512 : (qc + 1) * 512],
                            start=True, stop=True,
                        )
                        if qc % 2 == 0:
                            nc.vector.tensor_copy(
                                t[:, qc * 512 : (qc + 1) * 512], pps[:]
                            )
                        else:
                            nc.scalar.copy(t[:, qc * 512 : (qc + 1) * 512], pps[:])
                    dst_list.append(t)

            # ---- attention, one head-pair at a time -----------------------
            for p in range(HPC // 2):
                # pre-zeroed ctx accumulator; all ctx matmuls run with
                # start=False so accumulate-or-overwrite both land on the
                # matmul result over zeros regardless of the bank's stale
                # per-element has_written state (the two heads interleave
                # in one bank, so a start=True whole-bank clear would race)
                ctx_ps = ctxp.tile([128, S], F32, tag="ctx")
                nc.vector.memset(ctx_ps[:], 0.0)

                def emit_vn(kc, sub, zp_t, z_add, p=p):
                    # normalisation chain, one iteration ahead of its ctx
                    # matmuls: Pool add -> DVE recip -> Pool mul (Pool is
                    # otherwise idle; keeps big DVE ops off this path)
                    h = 2 * p + sub
                    if z_add:
                        z_t = small.tile([128, 1], F32, tag=f"z{sub}")
                        nc.vector.tensor_add(z_t[:], zp_t[:, 0:1], zp_t[:, 1:2])
                    else:
                        z_t = zp_t
                    zr_t = small.tile([128, 1], F32, tag=f"zr{sub}")
                    nc.vector.reciprocal(zr_t[:], z_t[:, 0:1])
                    vn_t = small.tile([128, D], BF16, tag=f"vn{sub}")
                    nc.vector.tensor_scalar_mul(
                        vn_t[:], v_sb[kc][:, h * D : (h + 1) * D], zr_t[:]
                    )
                    return vn_t

                def emit_ctx(sub, e_t, vn_t, ctx_ps=ctx_ps):
                    for qc in range(4):
                        nc.tensor.matmul(
                            ctx_ps[sub * D : (sub + 1) * D,
                                   qc * 512 : (qc + 1) * 512],
                            vn_t[:],
                            e_t[:, qc * 512 : (qc + 1) * 512],
                            start=False, stop=False,
                            skip_group_check=True,
                        )

                # pend[sub]: deque of (kc, e_t, zp_t, z_add) / vn results
                pend_z = {0: [], 1: []}   # awaiting vn chain (depth 1)
                pend_c = {0: [], 1: []}   # awaiting ctx matmuls (depth 1)
                for kc in range(KC):
                    for sub in (0, 1):
                        po = D * sub
                        use_hack = _use_hack(kc, sub)
                        # 1) scores + exp/hack for kc; the hack halves go
                        #    FIRST in the DVE queue so the score PSUM bank
                        #    frees as early as possible (PE's next scores
                        #    wait on it)
                        e_t = ep.tile([128, S], BF16, tag=f"e{sub}")
                        zp_t = small.tile([128, 2], F32, tag=f"zp{sub}")
                        for half in (0, 1):
                            s_t = scp[sub].tile([128, 1024], F32, tag=f"sc{sub}")
                            for qq in (0, 1):
                                j = half * 2 + qq
                                nc.tensor.matmul(
                                    s_t[:, qq * 512 : (qq + 1) * 512],
                                    kt_sb[p][po : po + D, kc * 128 : (kc + 1) * 128],
                                    qt_sb[p][po : po + D, j * 512 : (j + 1) * 512],
                                    start=True, stop=True,
                                )
                            esl = e_t[:, half * 1024 : (half + 1) * 1024]
                            if use_hack:
                                # exp(s/8) = 2^(s/8*log2e) via integer bits:
                                # i16 = s*(16*log2e) + (127 - C_SHIFT)*128,
                                # bitcast to bf16 (same exponent layout).
                                # Sawtooth rel err ~+-3% cancels in the ctx
                                # k-sum (verified 2.8e-3 end-to-end).
                                nc.vector.tensor_scalar(
                                    esl.bitcast(mybir.dt.int16), s_t[:],
                                    HACK_A, HACK_B,
                                    op0=mybir.AluOpType.mult,
                                    op1=mybir.AluOpType.add,
                                )
                            else:
                                # exp + z-sum in one ACT pass (accumulator
                                # read costs only ~280ns vs a 1x-mode DVE
                                # reduce at ~1.4us)
                                nc.scalar.activation(
                                    esl, s_t[:],
                                    mybir.ActivationFunctionType.Exp,
                                    scale=0.125,
                                    accum_out=zp_t[:, half : half + 1],
                                )
                        # 2) vn chain for kc-1 (z ready by now) -- after the
                        #    hack halves, before the bulky z-pass
                        if pend_z[sub]:
                            zkc, ze_t, zzp, zadd = pend_z[sub].pop(0)
                            vn_t = emit_vn(zkc, sub, zzp, zadd)
                            pend_c[sub].append((ze_t, vn_t))
                        # 3) z for hack chunks: tensor_scalar accum pass
                        #    (1x mode), last in the DVE queue -- only the
                        #    NEXT iteration's vn chain needs it
                        if use_hack:
                            nc.vector.tensor_scalar(
                                zscr[:], e_t[:], 1.0, None,
                                op0=mybir.AluOpType.mult,
                                op1=mybir.AluOpType.add,
                                accum_out=zp_t[:, 0:1],
                            )
                        pend_z[sub].append((kc, e_t, zp_t, not use_hack))
                    # 4) every 4 kc, flush the accumulated ctx matmuls as
                    #    one dense block: these are always-ready PE work
                    #    (vn computed >= 1 iteration ago) that fills the
                    #    PE's exp-wait bubbles and keeps the HAM power
                    #    governor from re-throttling the PE array
                    if kc % 4 == 3:
                        for sub in (0, 1):
                            while pend_c[sub]:
                                ce_t, cvn_t = pend_c[sub].pop(0)
                                emit_ctx(sub, ce_t, cvn_t)
                # drain the pipeline
                for sub in (0, 1):
                    while pend_z[sub] or pend_c[sub]:
                        if pend_z[sub]:
                            zkc, ze_t, zzp, zadd = pend_z[sub].pop(0)
                            vn_t = emit_vn(zkc, sub, zzp, zadd)
                            pend_c[sub].append((ze_t, vn_t))
                        ce_t, cvn_t = pend_c[sub].pop(0)
                        emit_ctx(sub, ce_t, cvn_t)

                ctx_sb = cs.tile([128, S], BF16, tag="ctx_sb")
                nc.vector.tensor_copy(ctx_sb[:], ctx_ps[:])

                for sub in (0, 1):
                    h = 2 * p + sub
                    po = D * sub
                    for qc in range(4):
                        o_ps = scp[sub].tile([D, 512], F32, tag=f"sc{sub}")
                        nc.tensor.matmul(
                            o_ps[:],
                            wo_pair[po : po + D, h, :],
                            ctx_sb[po : po + D, qc * 512 : (qc + 1) * 512],
                            start=True, stop=True,
                        )
                        sl = slice(qc * 512, (qc + 1) * 512)
                        if p == 0 and sub == 0:
                            nc.vector.tensor_copy(out_acc[:, sl], o_ps[:])
                        else:
                            nc.vector.tensor_add(
                                out_acc[:, sl], out_acc[:, sl], o_ps[:]
                            )

            # ---- epilogue: half-bias + store partial (pair-reduced by a
            # grouped psum fused into the same jitted dispatch) --------------
            nc.vector.tensor_scalar_add(out_acc[:], out_acc[:], bo_t[:])
            nc.gpsimd.dma_start(out=out_ext[:], in_=out_acc[:])

    _fix_drain_waits(nc)
    return nc


def shard_inputs(q, Wq, bq, Wk, bk, Wv, bv, Wo, bo):
    in_maps = []
    for c in range(N_CORES):
        b, j = c // 2, c % 2
        hs = slice(j * HB, (j + 1) * HB)
        in_maps.append(
            {
                "qt": np.ascontiguousarray(q[b].T, dtype=np.float32),
                "wq": np.ascontiguousarray(Wq[:, hs], dtype=np.float32),
                "bq": np.ascontiguousarray(bq[hs], dtype=np.float32),
                "wk": np.ascontiguousarray(Wk[:, hs], dtype=np.float32),
                "bk": np.ascontiguousarray(bk[hs], dtype=np.float32),
                "wv": np.ascontiguousarray(Wv[:, hs], dtype=np.float32),
                "bv": np.ascontiguousarray(bv[hs], dtype=np.float32),
                "wo": np.ascontiguousarray(Wo[hs, :], dtype=np.float32),
                "bo": np.ascontiguousarray(bo, dtype=np.float32),
            }
        )
    return in_maps


_CACHE = {}


def get_nc():
    if "nc" not in _CACHE:
        _CACHE["nc"] = _build()
    return _CACHE["nc"]


def run_spmd(nc, in_maps):
    """run_bass_via_pjrt with a grouped psum dispatched on-device right
    after the bass NEFF (the NEFF-embedded collective_compute hangs under
    this runtime, so the pair-reduction runs as an XLA collective; the
    bass_exec jit must contain only the custom call, so the psum is its
    own dispatch on device-resident outputs)."""
    import jax
    from jax.sharding import Mesh, PartitionSpec
    from jax.experimental.shard_map import shard_map
    from concourse import bass2jax

    bass2jax.install_neuronx_cc_hook()

    partition_name = nc.partition_id_tensor.name if nc.partition_id_tensor else None
    in_names, out_names, out_avals, zero_outs = [], [], [], []
    for alloc in nc.m.functions[0].allocations:
        if not isinstance(alloc, mybir.MemoryLocationSet):
            continue
        name = alloc.memorylocations[0].name
        if alloc.kind == "ExternalInput":
            if name != partition_name:
                in_names.append(name)
        elif alloc.kind == "ExternalOutput":
            out_names.append(name)
            shape = tuple(alloc.tensor_shape)
            dtype = mybir.dt.np(alloc.dtype)
            out_avals.append(jax.core.ShapedArray(shape, dtype))
            zero_outs.append(np.zeros(shape, dtype))
    n_params = len(in_names)
    n_outs = len(out_avals)
    in_names = in_names + out_names
    if partition_name is not None:
        in_names.append(partition_name)
    donate = tuple(range(n_params, n_params + n_outs))

    def _body(*args):
        operands = list(args)
        if partition_name is not None:
            operands.append(bass2jax.partition_id_tensor())
        outs = bass2jax._bass_exec_p.bind(
            *operands,
            out_avals=tuple(out_avals),
            in_names=tuple(in_names),
            out_names=tuple(out_names),
            lowering_input_output_aliases=(),
            sim_require_finite=True,
            sim_require_nnan=True,
            nc=nc,
        )
        return tuple(outs)

    devices = jax.devices()[:N_CORES]
    mesh = Mesh(np.asarray(devices), ("core",))
    sharded = jax.jit(
        shard_map(
            _body,
            mesh=mesh,
            in_specs=(PartitionSpec("core"),) * (n_params + n_outs),
            out_specs=(PartitionSpec("core"),) * n_outs,
            check_rep=False,
        ),
        donate_argnums=donate,
        keep_unused=True,
    )
    per_core = [[np.asarray(m[name]) for name in in_names[:n_params]] for m in in_maps]
    concat_in = [
        np.concatenate([per_core[c][i] for c in range(N_CORES)], axis=0)
        for i in range(n_params)
    ]
    concat_zeros = [
        np.zeros((N_CORES * z.shape[0], *z.shape[1:]), z.dtype) for z in zero_outs
    ]
    out_arrs = sharded(*concat_in, *concat_zeros)

    # pair-reduce on device: separate dispatch (the bass_exec jit must
    # contain only the custom call, per neuronx_cc_hook's checks)
    def _reduce(*outs):
        return tuple(
            jax.lax.psum(o, "core", axis_index_groups=REPLICA_GROUPS) for o in outs
        )

    reducer = jax.jit(
        shard_map(
            _reduce,
            mesh=mesh,
            in_specs=(PartitionSpec("core"),) * n_outs,
            out_specs=(PartitionSpec("core"),) * n_outs,
            check_rep=False,
        )
    )
    out_arrs = reducer(*out_arrs)
    return [
        {
            name: np.asarray(out_arrs[i]).reshape(N_CORES, *out_avals[i].shape)[c]
            for i, name in enumerate(out_names)
        }
        for c in range(N_CORES)
    ]


def kernel(q, Wq, bq, Wk, bk, Wv, bv, Wo, bo):
    nc = get_nc()
    in_maps = shard_inputs(q, Wq, bq, Wk, bk, Wv, bv, Wo, bo)
    results = run_spmd(nc, in_maps)
    out = np.stack([results[2 * b]["out"].T for b in range(B)], axis=0)
    return out.astype(np.float32)



# revision 36
# speedup vs baseline: 1.5762x; 1.2910x over previous
"""Distributed Trainium2 Bass kernel for nn_AttentionBlock_76115410419715.

Math (B=4, S=2048, D=64, H=12; softmax over the QUERY axis):
    qp = q@Wq+bq, kp = q@Wk+bk, vp = q@Wv+bv          (per-head blocks of 64)
    s[b,h,q,k] = qp . kp / 8
    attn = exp(s) / colsum_q(exp(s))                   [softmax over q]
    ctx[b,q,h,:] = sum_k attn[q,k] vp[k,:]
    out = ctx @ Wo + bo

Key identity used here: out = sum_h attn_h @ (vp_h @ Wo_h) + bo, so Wo is
folded into the V projection ON THE HOST (W2_h = Wv_h @ Wo_h, b2_h =
bv_h @ Wo_h) and the kernel computes m = q@W2+b2 once, then accumulates
    out^T[o, q] += sum_kc (m_kc[k, o]/z[k])^T @ e_kc[k, q]
directly in a single 2-bank PSUM region packed as [128, 1024]
(partitions 0:64 = q-half 0, 64:128 = q-half 1). No ctx tensor, no
separate out-projection, no per-pair PSUM drain.

Sharding: (batch, head-half) across 8 cores -- core c handles batch c//2
and heads [6*(c%2), 6*(c%2)+6). A grouped psum over core pairs {2b, 2b+1}
(dispatched on-device right after the bass NEFF) produces the full output
for batch b (each core adds bo/2).

Per-core pipeline, scores never hit HBM:
  - projections with bias folded in via an appended ones-row
  - per (kc, sub): 4 score matmuls ([128,1024] f32 PSUM halves from a
    3-deep rotating pool = 6 banks) -> exp on ACT (with accumulator z) or
    the i16-bits exp hack on DVE (z via a 1x accum pass) -> reciprocal ->
    mn = m*zr -> 4 out-matmuls, batched in 4-kc blocks of always-ready PE
    work to keep the HAM power governor warm.
"""

import sys

if "/opt/trn_rl_repo" not in sys.path:
    sys.path.insert(0, "/opt/trn_rl_repo")

import numpy as np

import concourse.bass as bass
import concourse.tile as tile
from concourse import mybir

B, S, D, H = 4, 2048, 64, 12
N_CORES = 8
HPC = 6          # heads per core
HB = HPC * D     # 384, per-core head-block width
KC = S // 128    # 16 k-chunks
F32 = mybir.dt.float32
BF16 = mybir.dt.bfloat16
REPLICA_GROUPS = [[0, 1], [2, 3], [4, 5], [6, 7]]

# i16-bits exp: exp(s/8) = 2^y, y = s*0.125*log2(e); i16 = y*128 + HACK_B
# then bitcast to bf16 (8-bit exponent, 7-bit mantissa -> (1+frac)*2^floor).
C_SHIFT = 0.043  # centers the (1+f)/2^f sawtooth (rel err to +-3%)
HACK_A = 0.125 * np.log2(np.e) * 128.0
HACK_B = (127.0 - C_SHIFT) * 128.0


# kc indices (staggered per sub) whose exp runs as the DVE hack instead of
# ACT exp -- the ACT/DVE load-balance knob (4 of 16 kc per sub = 24 chunks)
def _use_hack(kc, sub):
    return (kc + 2 * sub) % 4 == 3


def _fix_drain_waits(nc):
    """This walrus build rejects instructions carrying >1 sem wait; move
    extras onto same-engine NOPs inserted immediately before (same engine
    stream => identical blocking semantics)."""
    eng = {
        mybir.EngineType.SP: nc.sync,
        mybir.EngineType.Pool: nc.gpsimd,
        mybir.EngineType.DVE: nc.vector,
        mybir.EngineType.Activation: nc.scalar,
        mybir.EngineType.PE: nc.tensor,
    }
    for bb in nc.main_func.blocks:
        fixes = []
        for idx, ins in enumerate(bb.instructions):
            si = ins.sync_info
            if (
                si is not None
                and si.on_wait is not None
                and len(si.on_wait) > 1
                and ins.engine in eng
            ):
                fixes.append((idx, ins))
        for idx, ins in reversed(fixes):
            si = ins.sync_info
            waits = list(si.on_wait)
            si.on_wait[:] = waits[-1:]
            nops = []
            for w in waits[:-1]:
                bi = eng[ins.engine].nop(nofuse=True, hint="split_wait")
                nop_ins = bi.ins
                for bb2 in nc.main_func.blocks:
                    if nop_ins in bb2.instructions:
                        bb2.instructions.remove(nop_ins)
                        break
                nsi = nop_ins.sync_info
                if nsi is None:
                    nop_ins.sync_info = type(si)(on_wait=[w], on_update=[])
                else:
                    nsi.on_wait[:] = [w]
                nops.append(nop_ins)
            for j, nop_ins in enumerate(nops):
                bb.instructions.insert(idx + j, nop_ins)


def _build():
    nc = bass.Bass(num_devices=N_CORES)

    qt_ext = nc.declare_dram_parameter("qt", [D, S], F32, isOutput=False)
    wq_ext = nc.declare_dram_parameter("wq", [D, HB], F32, isOutput=False)
    bq_ext = nc.declare_dram_parameter("bq", [HB], F32, isOutput=False)
    wk_ext = nc.declare_dram_parameter("wk", [D, HB], F32, isOutput=False)
    bk_ext = nc.declare_dram_parameter("bk", [HB], F32, isOutput=False)
    wv_ext = nc.declare_dram_parameter("wv", [D, HB], F32, isOutput=False)
    bv_ext = nc.declare_dram_parameter("bv", [HB], F32, isOutput=False)
    bo_ext = nc.declare_dram_parameter("bo", [D], F32, isOutput=False)
    out_ext = nc.declare_dram_parameter("out", [D, S], F32, isOutput=True)

    with tile.TileContext(nc) as tc:
        with (
            tc.tile_pool(name="const", bufs=1) as const,
            tc.tile_pool(name="ld", bufs=2) as ld,
            tc.tile_pool(name="qk", bufs=1) as qk,
            tc.tile_pool(name="mp", bufs=1) as mpool,
            tc.tile_pool(name="ep", bufs=6) as ep,
            tc.tile_pool(name="small", bufs=8) as small,
            tc.tile_pool(name="scp", bufs=3, space="PSUM") as scp,
            tc.tile_pool(name="outp", bufs=1, space="PSUM") as outp,
        ):
            # ---- load + prep constants -------------------------------------
            qte = const.tile([D + 1, S], BF16, tag="qte")
            qt_f32 = ld.tile([D, S], F32, tag="ldq")
            nc.gpsimd.dma_start(out=qt_f32[:], in_=qt_ext[:])
            nc.vector.tensor_copy(qte[0:D, :], qt_f32[:])
            nc.vector.memset(qte[D : D + 1, :], 1.0)

            def load_we(w_ext, b_ext, tag):
                we = const.tile([D + 1, HB], BF16, tag=tag)
                w_f32 = ld.tile([D, HB], F32, tag="ldw")
                nc.gpsimd.dma_start(out=w_f32[:], in_=w_ext[:])
                nc.vector.tensor_copy(we[0:D, :], w_f32[:])
                b_f32 = ld.tile([1, HB], F32, tag="ldb")
                nc.gpsimd.dma_start(
                    out=b_f32[:], in_=b_ext.rearrange("(a b) -> a b", a=1)
                )
                nc.vector.tensor_copy(we[D : D + 1, :], b_f32[:])
                return we

            wq_e = load_we(wq_ext, bq_ext, "wq")
            wk_e = load_we(wk_ext, bk_ext, "wk")
            wv_e = load_we(wv_ext, bv_ext, "wv")  # host-folded Wv@Wo / bv@Wo

            # bo/2 duplicated on both partition halves (the packed out
            # region holds q-half 0 on partitions 0:64, q-half 1 on 64:128;
            # each core of a psum pair adds half the bias)
            bo2_t = const.tile([128, 1], F32, tag="bo")
            nc.gpsimd.dma_start(
                out=bo2_t[0:D, :], in_=bo_ext.rearrange("(a b) -> a b", b=1)
            )
            nc.gpsimd.dma_start(
                out=bo2_t[D : 2 * D, :],
                in_=bo_ext.rearrange("(a b) -> a b", b=1),
            )
            nc.vector.tensor_scalar_mul(bo2_t[:], bo2_t[:], 0.5)

            # write-only scratch for the hack-chunk z-sum (accum output)
            zscr = const.tile([128, S], BF16, tag="zscr")

            # persistent packed out^T accumulator: [0:64, :] = q 0:1024,
            # [64:128, :] = q 1024:2048; all heads/kc accumulate here
            out_ps = outp.tile([128, 1024], F32, tag="out")
            nc.vector.memset(out_ps[:], 0.0)

            # ---- projections ----------------------------------------------
            # m = q@(Wv Wo) + (bv Wo), natural layout: m_sb[sc][s(128), HB]
            m_sb = []
            for sc in range(KC):
                m_ps = scp.tile([128, HB], F32, tag="sc")
                nc.tensor.matmul(
                    m_ps[:], qte[:, sc * 128 : (sc + 1) * 128], wv_e[:],
                    start=True, stop=True,
                )
                mt = mpool.tile([128, HB], BF16, tag=f"m{sc}")
                if sc % 2 == 0:
                    nc.vector.tensor_copy(mt[:], m_ps[:])
                else:
                    nc.scalar.copy(mt[:], m_ps[:])
                m_sb.append(mt)

            # Q^T / K^T: per head-pair tiles [128 (2 heads x 64 dout), S]
            qt_sb, kt_sb = [], []
            for p in range(HPC // 2):
                for (we, dst_list, tg) in ((wq_e, qt_sb, "q"), (wk_e, kt_sb, "k")):
                    t = qk.tile([128, S], BF16, tag=f"{tg}{p}")
                    for qc in range(4):
                        pps = scp.tile([128, 512], F32, tag="sc")
                        nc.tensor.matmul(
                            pps[:],
                            we[:, p * 128 : (p + 1) * 128],
                            qte[:, qc * 512 : (qc + 1) * 512],
                            start=True, stop=True,
                        )
                        if qc % 2 == 0:
                            nc.vector.tensor_copy(
                                t[:, qc * 512 : (qc + 1) * 512], pps[:]
                            )
                        else:
                            nc.scalar.copy(t[:, qc * 512 : (qc + 1) * 512], pps[:])
                    dst_list.append(t)

            # ---- attention + output accumulation, one head-pair at a time --
            for p in range(HPC // 2):

                def emit_mn(kc, sub, zp_t, z_add, p=p):
                    # normalisation chain (DVE), one iteration ahead of its
                    # out-matmuls: z -> 1/z -> mn = m/z
                    h = 2 * p + sub
                    if z_add:
                        z_t = small.tile([128, 1], F32, tag=f"z{sub}")
                        nc.vector.tensor_add(z_t[:], zp_t[:, 0:1], zp_t[:, 1:2])
                    else:
                        z_t = zp_t
                    zr_t = small.tile([128, 1], F32, tag=f"zr{sub}")
                    nc.vector.reciprocal(zr_t[:], z_t[:, 0:1])
                    mn_t = small.tile([128, D], BF16, tag=f"mn{sub}")
                    nc.vector.tensor_scalar_mul(
                        mn_t[:], m_sb[kc][:, h * D : (h + 1) * D], zr_t[:]
                    )
                    return mn_t

                def emit_out(e_t, mn_t, out_ps=out_ps):
                    # out^T[o, q] += mn^T @ e, packed: q-half = partition
                    # half. start=False over the pre-zeroed region (stale
                    # has_written state is irrelevant: always accumulate)
                    for qc in range(4):
                        nc.tensor.matmul(
                            out_ps[(qc // 2) * D : (qc // 2 + 1) * D,
                                   (qc % 2) * 512 : (qc % 2 + 1) * 512],
                            mn_t[:],
                            e_t[:, qc * 512 : (qc + 1) * 512],
                            start=False, stop=False,
                            skip_group_check=True,
                        )

                pend_z = {0: [], 1: []}   # chunks awaiting the mn chain
                pend_c = {0: [], 1: []}   # chunks awaiting out-matmuls
                for kc in range(KC):
                    for sub in (0, 1):
                        po = D * sub
                        use_hack = _use_hack(kc, sub)
                        # 1) scores + exp/hack for kc; the hack goes first
                        #    in the DVE queue so the score PSUM bank frees
                        #    as early as possible
                        e_t = ep.tile([128, S], BF16, tag=f"e{sub}")
                        zp_t = small.tile([128, 2], F32, tag=f"zp{sub}")
                        for half in (0, 1):
                            s_t = scp.tile([128, 1024], F32, tag="sc")
                            for qq in (0, 1):
                                j = half * 2 + qq
                                nc.tensor.matmul(
                                    s_t[:, qq * 512 : (qq + 1) * 512],
                                    kt_sb[p][po : po + D, kc * 128 : (kc + 1) * 128],
                                    qt_sb[p][po : po + D, j * 512 : (j + 1) * 512],
                                    start=True, stop=True,
                                )
                            esl = e_t[:, half * 1024 : (half + 1) * 1024]
                            if use_hack:
                                # exp(s/8) = 2^(s/8*log2e) via integer bits:
                                # i16 = s*(16*log2e) + (127 - C_SHIFT)*128,
                                # bitcast to bf16 (same exponent layout).
                                # Sawtooth rel err ~+-3% cancels in the
                                # out-matmul k-sum (2.8e-3 end-to-end).
                                nc.vector.tensor_scalar(
                                    esl.bitcast(mybir.dt.int16), s_t[:],
                                    HACK_A, HACK_B,
                                    op0=mybir.AluOpType.mult,
                                    op1=mybir.AluOpType.add,
                                )
                            else:
                                # exp + z-sum in one ACT pass (accumulator
                                # read costs only ~285ns vs a 1x-mode DVE
                                # reduce at ~2.3us)
                                nc.scalar.activation(
                                    esl, s_t[:],
                                    mybir.ActivationFunctionType.Exp,
                                    scale=0.125,
                                    accum_out=zp_t[:, half : half + 1],
                                )
                        # 2) mn chain for kc-1 (z ready by now)
                        if pend_z[sub]:
                            zkc, ze_t, zzp, zadd = pend_z[sub].pop(0)
                            mn_t = emit_mn(zkc, sub, zzp, zadd)
                            pend_c[sub].append((ze_t, mn_t))
                        # 3) z for hack chunks: tensor_scalar accum pass
                        #    (1x mode), last in the DVE queue -- only the
                        #    NEXT iteration's mn chain needs it
                        if use_hack:
                            nc.vector.tensor_scalar(
                                zscr[:], e_t[:], 1.0, None,
                                op0=mybir.AluOpType.mult,
                                op1=mybir.AluOpType.add,
                                accum_out=zp_t[:, 0:1],
                            )
                        pend_z[sub].append((kc, e_t, zp_t, not use_hack))
                    # 4) every 4 kc, flush the accumulated out-matmuls as
                    #    one dense block: always-ready PE work (mn computed
                    #    >= 1 iteration ago) that fills the PE's exp-wait
                    #    bubbles and keeps the HAM power governor warm
                    if kc % 4 == 3:
                        for sub in (0, 1):
                            while pend_c[sub]:
                                ce_t, cmn_t = pend_c[sub].pop(0)
                                emit_out(ce_t, cmn_t)
                # drain the pair pipeline
                for sub in (0, 1):
                    while pend_z[sub] or pend_c[sub]:
                        if pend_z[sub]:
                            zkc, ze_t, zzp, zadd = pend_z[sub].pop(0)
                            mn_t = emit_mn(zkc, sub, zzp, zadd)
                            pend_c[sub].append((ze_t, mn_t))
                        ce_t, cmn_t = pend_c[sub].pop(0)
                        emit_out(ce_t, cmn_t)

            # ---- epilogue: half-bias + store partial (pair-reduced by a
            # grouped psum fused into the same jitted dispatch) --------------
            out_sb = const.tile([128, 1024], F32, tag="out_sb")
            nc.vector.tensor_scalar_add(out_sb[:], out_ps[:], bo2_t[:])
            nc.gpsimd.dma_start(out=out_ext[:, 0:1024], in_=out_sb[0:D, :])
            nc.gpsimd.dma_start(
                out=out_ext[:, 1024:2048], in_=out_sb[D : 2 * D, :]
            )

    _fix_drain_waits(nc)
    return nc


def shard_inputs(q, Wq, bq, Wk, bk, Wv, bv, Wo, bo):
    in_maps = []
    for c in range(N_CORES):
        b, j = c // 2, c % 2
        hs = slice(j * HB, (j + 1) * HB)
        # fold Wo into the V projection per head: W2_h = Wv_h @ Wo_h,
        # b2_h = bv_h @ Wo_h  (out = sum_h attn_h @ vp_h @ Wo_h + bo)
        Wv_c = np.asarray(Wv[:, hs], dtype=np.float64)
        bv_c = np.asarray(bv[hs], dtype=np.float64)
        Wo_c = np.asarray(Wo[hs, :], dtype=np.float64)
        W2 = np.empty((D, HB), dtype=np.float64)
        b2 = np.empty((HB,), dtype=np.float64)
        for l in range(HPC):
            blk = slice(l * D, (l + 1) * D)
            W2[:, blk] = Wv_c[:, blk] @ Wo_c[blk, :]
            b2[blk] = bv_c[blk] @ Wo_c[blk, :]
        in_maps.append(
            {
                "qt": np.ascontiguousarray(q[b].T, dtype=np.float32),
                "wq": np.ascontiguousarray(Wq[:, hs], dtype=np.float32),
                "bq": np.ascontiguousarray(bq[hs], dtype=np.float32),
                "wk": np.ascontiguousarray(Wk[:, hs], dtype=np.float32),
                "bk": np.ascontiguousarray(bk[hs], dtype=np.float32),
                "wv": W2.astype(np.float32),
                "bv": b2.astype(np.float32),
                "bo": np.ascontiguousarray(bo, dtype=np.float32),
            }
        )
    return in_maps


_CACHE = {}


def get_nc():
    if "nc" not in _CACHE:
        _CACHE["nc"] = _build()
    return _CACHE["nc"]


def run_spmd(nc, in_maps):
    """run_bass_via_pjrt with a grouped psum dispatched on-device right
    after the bass NEFF (the NEFF-embedded collective_compute hangs under
    this runtime, so the pair-reduction runs as an XLA collective; the
    bass_exec jit must contain only the custom call, so the psum is its
    own dispatch on device-resident outputs)."""
    import jax
    from jax.sharding import Mesh, PartitionSpec
    from jax.experimental.shard_map import shard_map
    from concourse import bass2jax

    bass2jax.install_neuronx_cc_hook()

    partition_name = nc.partition_id_tensor.name if nc.partition_id_tensor else None
    in_names, out_names, out_avals, zero_outs = [], [], [], []
    for alloc in nc.m.functions[0].allocations:
        if not isinstance(alloc, mybir.MemoryLocationSet):
            continue
        name = alloc.memorylocations[0].name
        if alloc.kind == "ExternalInput":
            if name != partition_name:
                in_names.append(name)
        elif alloc.kind == "ExternalOutput":
            out_names.append(name)
            shape = tuple(alloc.tensor_shape)
            dtype = mybir.dt.np(alloc.dtype)
            out_avals.append(jax.core.ShapedArray(shape, dtype))
            zero_outs.append(np.zeros(shape, dtype))
    n_params = len(in_names)
    n_outs = len(out_avals)
    in_names = in_names + out_names
    if partition_name is not None:
        in_names.append(partition_name)
    donate = tuple(range(n_params, n_params + n_outs))

    def _body(*args):
        operands = list(args)
        if partition_name is not None:
            operands.append(bass2jax.partition_id_tensor())
        outs = bass2jax._bass_exec_p.bind(
            *operands,
            out_avals=tuple(out_avals),
            in_names=tuple(in_names),
            out_names=tuple(out_names),
            lowering_input_output_aliases=(),
            sim_require_finite=True,
            sim_require_nnan=True,
            nc=nc,
        )
        return tuple(outs)

    devices = jax.devices()[:N_CORES]
    mesh = Mesh(np.asarray(devices), ("core",))
    sharded = jax.jit(
        shard_map(
            _body,
            mesh=mesh,
            in_specs=(PartitionSpec("core"),) * (n_params + n_outs),
            out_specs=(PartitionSpec("core"),) * n_outs,
            check_rep=False,
        ),
        donate_argnums=donate,
        keep_unused=True,
    )
    per_core = [[np.asarray(m[name]) for name in in_names[:n_params]] for m in in_maps]
    concat_in = [
        np.concatenate([per_core[c][i] for c in range(N_CORES)], axis=0)
        for i in range(n_params)
    ]
    concat_zeros = [
        np.zeros((N_CORES * z.shape[0], *z.shape[1:]), z.dtype) for z in zero_outs
    ]
    out_arrs = sharded(*concat_in, *concat_zeros)

    # pair-reduce on device: separate dispatch (the bass_exec jit must
    # contain only the custom call, per neuronx_cc_hook's checks)
    def _reduce(*outs):
        return tuple(
            jax.lax.psum(o, "core", axis_index_groups=REPLICA_GROUPS) for o in outs
        )

    reducer = jax.jit(
        shard_map(
            _reduce,
            mesh=mesh,
            in_specs=(PartitionSpec("core"),) * n_outs,
            out_specs=(PartitionSpec("core"),) * n_outs,
            check_rep=False,
        )
    )
    out_arrs = reducer(*out_arrs)
    return [
        {
            name: np.asarray(out_arrs[i]).reshape(N_CORES, *out_avals[i].shape)[c]
            for i, name in enumerate(out_names)
        }
        for c in range(N_CORES)
    ]


def kernel(q, Wq, bq, Wk, bk, Wv, bv, Wo, bo):
    nc = get_nc()
    in_maps = shard_inputs(q, Wq, bq, Wk, bk, Wv, bv, Wo, bo)
    results = run_spmd(nc, in_maps)
    out = np.stack([results[2 * b]["out"].T for b in range(B)], axis=0)
    return out.astype(np.float32)


# revision 37
# speedup vs baseline: 1.5922x; 1.0102x over previous
"""Distributed Trainium2 Bass kernel for nn_AttentionBlock_76115410419715.

Math (B=4, S=2048, D=64, H=12; softmax over the QUERY axis):
    qp = q@Wq+bq, kp = q@Wk+bk, vp = q@Wv+bv          (per-head blocks of 64)
    s[b,h,q,k] = qp . kp / 8
    attn = exp(s) / colsum_q(exp(s))                   [softmax over q]
    ctx[b,q,h,:] = sum_k attn[q,k] vp[k,:]
    out = ctx @ Wo + bo

Key identity used here: out = sum_h attn_h @ (vp_h @ Wo_h) + bo, so Wo is
folded into the V projection ON THE HOST (W2_h = Wv_h @ Wo_h, b2_h =
bv_h @ Wo_h) and the kernel computes m = q@W2+b2 once, then accumulates
    out^T[o, q] += sum_kc (m_kc[k, o]/z[k])^T @ e_kc[k, q]
directly in a single 2-bank PSUM region packed as [128, 1024]
(partitions 0:64 = q-half 0, 64:128 = q-half 1). No ctx tensor, no
separate out-projection, no per-pair PSUM drain.

Sharding: (batch, head-half) across 8 cores -- core c handles batch c//2
and heads [6*(c%2), 6*(c%2)+6). A grouped psum over core pairs {2b, 2b+1}
(dispatched on-device right after the bass NEFF) produces the full output
for batch b (each core adds bo/2).

Per-core pipeline, scores never hit HBM:
  - projections with bias folded in via an appended ones-row
  - per (kc, sub): 4 score matmuls ([128,1024] f32 PSUM halves from a
    3-deep rotating pool = 6 banks) -> exp on ACT (with accumulator z) or
    the i16-bits exp hack on DVE (z via a 1x accum pass) -> reciprocal ->
    mn = m*zr -> 4 out-matmuls, batched in 4-kc blocks of always-ready PE
    work to keep the HAM power governor warm.
"""

import sys

if "/opt/trn_rl_repo" not in sys.path:
    sys.path.insert(0, "/opt/trn_rl_repo")

import numpy as np

import concourse.bass as bass
import concourse.tile as tile
from concourse import mybir

B, S, D, H = 4, 2048, 64, 12
N_CORES = 8
HPC = 6          # heads per core
HB = HPC * D     # 384, per-core head-block width
KC = S // 128    # 16 k-chunks
F32 = mybir.dt.float32
BF16 = mybir.dt.bfloat16
REPLICA_GROUPS = [[0, 1], [2, 3], [4, 5], [6, 7]]

# i16-bits exp: exp(s/8) = 2^y, y = s*0.125*log2(e); i16 = y*128 + HACK_B
# then bitcast to bf16 (8-bit exponent, 7-bit mantissa -> (1+frac)*2^floor).
C_SHIFT = 0.043  # centers the (1+f)/2^f sawtooth (rel err to +-3%)
HACK_A = 0.125 * np.log2(np.e) * 128.0
HACK_B = (127.0 - C_SHIFT) * 128.0


# kc indices (staggered per sub) whose exp runs as the DVE hack instead of
# ACT exp -- the ACT/DVE load-balance knob (4 of 16 kc per sub = 24 chunks)
def _use_hack(kc, sub):
    return (kc + 2 * sub) % 4 == 3


def _fix_drain_waits(nc):
    """This walrus build rejects instructions carrying >1 sem wait; move
    extras onto same-engine NOPs inserted immediately before (same engine
    stream => identical blocking semantics)."""
    eng = {
        mybir.EngineType.SP: nc.sync,
        mybir.EngineType.Pool: nc.gpsimd,
        mybir.EngineType.DVE: nc.vector,
        mybir.EngineType.Activation: nc.scalar,
        mybir.EngineType.PE: nc.tensor,
    }
    for bb in nc.main_func.blocks:
        fixes = []
        for idx, ins in enumerate(bb.instructions):
            si = ins.sync_info
            if (
                si is not None
                and si.on_wait is not None
                and len(si.on_wait) > 1
                and ins.engine in eng
            ):
                fixes.append((idx, ins))
        for idx, ins in reversed(fixes):
            si = ins.sync_info
            waits = list(si.on_wait)
            si.on_wait[:] = waits[-1:]
            nops = []
            for w in waits[:-1]:
                bi = eng[ins.engine].nop(nofuse=True, hint="split_wait")
                nop_ins = bi.ins
                for bb2 in nc.main_func.blocks:
                    if nop_ins in bb2.instructions:
                        bb2.instructions.remove(nop_ins)
                        break
                nsi = nop_ins.sync_info
                if nsi is None:
                    nop_ins.sync_info = type(si)(on_wait=[w], on_update=[])
                else:
                    nsi.on_wait[:] = [w]
                nops.append(nop_ins)
            for j, nop_ins in enumerate(nops):
                bb.instructions.insert(idx + j, nop_ins)


def _build():
    nc = bass.Bass(num_devices=N_CORES)

    qt_ext = nc.declare_dram_parameter("qt", [D, S], F32, isOutput=False)
    wq_ext = nc.declare_dram_parameter("wq", [D, HB], F32, isOutput=False)
    bq_ext = nc.declare_dram_parameter("bq", [HB], F32, isOutput=False)
    wk_ext = nc.declare_dram_parameter("wk", [D, HB], F32, isOutput=False)
    bk_ext = nc.declare_dram_parameter("bk", [HB], F32, isOutput=False)
    wv_ext = nc.declare_dram_parameter("wv", [D, HB], F32, isOutput=False)
    bv_ext = nc.declare_dram_parameter("bv", [HB], F32, isOutput=False)
    bo_ext = nc.declare_dram_parameter("bo", [D], F32, isOutput=False)
    out_ext = nc.declare_dram_parameter("out", [D, S], F32, isOutput=True)

    with tile.TileContext(nc) as tc:
        with (
            tc.tile_pool(name="const", bufs=1) as const,
            tc.tile_pool(name="ld", bufs=2) as ld,
            tc.tile_pool(name="qk", bufs=1) as qk,
            tc.tile_pool(name="mp", bufs=1) as mpool,
            tc.tile_pool(name="ep", bufs=6) as ep,
            tc.tile_pool(name="small", bufs=8) as small,
            tc.tile_pool(name="scp", bufs=3, space="PSUM") as scp,
            tc.tile_pool(name="outp", bufs=1, space="PSUM") as outp,
        ):
            # ---- load + prep constants -------------------------------------
            qte = const.tile([D + 1, S], BF16, tag="qte")
            qt_f32 = ld.tile([D, S], F32, tag="ldq")
            nc.gpsimd.dma_start(out=qt_f32[:], in_=qt_ext[:])
            nc.vector.tensor_copy(qte[0:D, :], qt_f32[:])
            nc.vector.memset(qte[D : D + 1, :], 1.0)

            def load_we(w_ext, b_ext, tag):
                we = const.tile([D + 1, HB], BF16, tag=tag)
                w_f32 = ld.tile([D, HB], F32, tag="ldw")
                nc.gpsimd.dma_start(out=w_f32[:], in_=w_ext[:])
                nc.vector.tensor_copy(we[0:D, :], w_f32[:])
                b_f32 = ld.tile([1, HB], F32, tag="ldb")
                nc.gpsimd.dma_start(
                    out=b_f32[:], in_=b_ext.rearrange("(a b) -> a b", a=1)
                )
                nc.vector.tensor_copy(we[D : D + 1, :], b_f32[:])
                return we

            wq_e = load_we(wq_ext, bq_ext, "wq")
            wk_e = load_we(wk_ext, bk_ext, "wk")
            wv_e = load_we(wv_ext, bv_ext, "wv")  # host-folded Wv@Wo / bv@Wo

            # bo/2 duplicated on both partition halves (the packed out
            # region holds q-half 0 on partitions 0:64, q-half 1 on 64:128;
            # each core of a psum pair adds half the bias)
            bo2_t = const.tile([128, 1], F32, tag="bo")
            nc.gpsimd.dma_start(
                out=bo2_t[0:D, :], in_=bo_ext.rearrange("(a b) -> a b", b=1)
            )
            nc.gpsimd.dma_start(
                out=bo2_t[D : 2 * D, :],
                in_=bo_ext.rearrange("(a b) -> a b", b=1),
            )
            nc.vector.tensor_scalar_mul(bo2_t[:], bo2_t[:], 0.5)

            # write-only scratch for the hack-chunk z-sum (accum output)
            zscr = const.tile([128, S], BF16, tag="zscr")

            # persistent packed out^T accumulator: [0:64, :] = q 0:1024,
            # [64:128, :] = q 1024:2048; all heads/kc accumulate here
            out_ps = outp.tile([128, 1024], F32, tag="out")
            nc.vector.memset(out_ps[:], 0.0)

            # ---- projections ----------------------------------------------
            # m = q@(Wv Wo) + (bv Wo), natural layout: m_sb[sc][s(128), HB]
            m_sb = []
            for sc in range(KC):
                m_ps = scp.tile([128, HB], F32, tag="sc")
                nc.tensor.matmul(
                    m_ps[:], qte[:, sc * 128 : (sc + 1) * 128], wv_e[:],
                    start=True, stop=True,
                )
                mt = mpool.tile([128, HB], BF16, tag=f"m{sc}")
                if sc % 2 == 0:
                    nc.vector.tensor_copy(mt[:], m_ps[:])
                else:
                    nc.scalar.copy(mt[:], m_ps[:])
                m_sb.append(mt)

            # Q^T / K^T: per head-pair tiles [128 (2 heads x 64 dout), S]
            qt_sb, kt_sb = [], []
            for p in range(HPC // 2):
                for (we, dst_list, tg) in ((wq_e, qt_sb, "q"), (wk_e, kt_sb, "k")):
                    t = qk.tile([128, S], BF16, tag=f"{tg}{p}")
                    for qc in range(4):
                        pps = scp.tile([128, 512], F32, tag="sc")
                        nc.tensor.matmul(
                            pps[:],
                            we[:, p * 128 : (p + 1) * 128],
                            qte[:, qc * 512 : (qc + 1) * 512],
                            start=True, stop=True,
                        )
                        if qc % 2 == 0:
                            nc.vector.tensor_copy(
                                t[:, qc * 512 : (qc + 1) * 512], pps[:]
                            )
                        else:
                            nc.scalar.copy(t[:, qc * 512 : (qc + 1) * 512], pps[:])
                    dst_list.append(t)

            # ---- attention + output accumulation, one head-pair at a time --
            for p in range(HPC // 2):

                def emit_mn(kc, sub, zp_t, z_add, p=p):
                    # normalisation chain (DVE), one iteration ahead of its
                    # out-matmuls: z -> 1/z -> mn = m/z
                    h = 2 * p + sub
                    if z_add:
                        z_t = small.tile([128, 1], F32, tag=f"z{sub}")
                        nc.vector.tensor_add(z_t[:], zp_t[:, 0:1], zp_t[:, 1:2])
                    else:
                        z_t = zp_t
                    zr_t = small.tile([128, 1], F32, tag=f"zr{sub}")
                    nc.vector.reciprocal(zr_t[:], z_t[:, 0:1])
                    mn_t = small.tile([128, D], BF16, tag=f"mn{sub}")
                    nc.vector.tensor_scalar_mul(
                        mn_t[:], m_sb[kc][:, h * D : (h + 1) * D], zr_t[:]
                    )
                    return mn_t

                def emit_out(e_t, mn_t, out_ps=out_ps):
                    # out^T[o, q] += mn^T @ e, packed: q-half = partition
                    # half. start=False over the pre-zeroed region (stale
                    # has_written state is irrelevant: always accumulate).
                    # qc order 0,2,1,3 alternates PE column groups so
                    # adjacent matmuls can stream concurrently
                    for qc in (0, 2, 1, 3):
                        nc.tensor.matmul(
                            out_ps[(qc // 2) * D : (qc // 2 + 1) * D,
                                   (qc % 2) * 512 : (qc % 2 + 1) * 512],
                            mn_t[:],
                            e_t[:, qc * 512 : (qc + 1) * 512],
                            start=False, stop=False,
                            skip_group_check=True,
                        )

                pend_z = {0: [], 1: []}   # chunks awaiting the mn chain
                pend_c = {0: [], 1: []}   # chunks awaiting out-matmuls
                for kc in range(KC):
                    for sub in (0, 1):
                        po = D * sub
                        use_hack = _use_hack(kc, sub)
                        # 1) scores + exp/hack for kc; the hack goes first
                        #    in the DVE queue so the score PSUM bank frees
                        #    as early as possible
                        e_t = ep.tile([128, S], BF16, tag=f"e{sub}")
                        zp_t = small.tile([128, 2], F32, tag=f"zp{sub}")
                        for half in (0, 1):
                            s_t = scp.tile([128, 1024], F32, tag="sc")
                            for qq in (0, 1):
                                j = half * 2 + qq
                                nc.tensor.matmul(
                                    s_t[:, qq * 512 : (qq + 1) * 512],
                                    kt_sb[p][po : po + D, kc * 128 : (kc + 1) * 128],
                                    qt_sb[p][po : po + D, j * 512 : (j + 1) * 512],
                                    start=True, stop=True,
                                )
                            esl = e_t[:, half * 1024 : (half + 1) * 1024]
                            if use_hack:
                                # exp(s/8) = 2^(s/8*log2e) via integer bits:
                                # i16 = s*(16*log2e) + (127 - C_SHIFT)*128,
                                # bitcast to bf16 (same exponent layout).
                                # Sawtooth rel err ~+-3% cancels in the
                                # out-matmul k-sum (2.8e-3 end-to-end).
                                nc.vector.tensor_scalar(
                                    esl.bitcast(mybir.dt.int16), s_t[:],
                                    HACK_A, HACK_B,
                                    op0=mybir.AluOpType.mult,
                                    op1=mybir.AluOpType.add,
                                )
                            else:
                                # exp + z-sum in one ACT pass (accumulator
                                # read costs only ~285ns vs a 1x-mode DVE
                                # reduce at ~2.3us)
                                nc.scalar.activation(
                                    esl, s_t[:],
                                    mybir.ActivationFunctionType.Exp,
                                    scale=0.125,
                                    accum_out=zp_t[:, half : half + 1],
                                )
                        # 2) mn chain for kc-1 (z ready by now)
                        if pend_z[sub]:
                            zkc, ze_t, zzp, zadd = pend_z[sub].pop(0)
                            mn_t = emit_mn(zkc, sub, zzp, zadd)
                            pend_c[sub].append((ze_t, mn_t))
                        # 3) z for hack chunks: tensor_scalar accum pass
                        #    (1x mode), last in the DVE queue -- only the
                        #    NEXT iteration's mn chain needs it
                        if use_hack:
                            nc.vector.tensor_scalar(
                                zscr[:], e_t[:], 1.0, None,
                                op0=mybir.AluOpType.mult,
                                op1=mybir.AluOpType.add,
                                accum_out=zp_t[:, 0:1],
                            )
                        pend_z[sub].append((kc, e_t, zp_t, not use_hack))
                    # 4) every 4 kc, flush the accumulated out-matmuls as
                    #    one dense block: always-ready PE work (mn computed
                    #    >= 1 iteration ago) that fills the PE's exp-wait
                    #    bubbles and keeps the HAM power governor warm
                    if kc % 4 == 3:
                        for sub in (0, 1):
                            while pend_c[sub]:
                                ce_t, cmn_t = pend_c[sub].pop(0)
                                emit_out(ce_t, cmn_t)
                # drain the pair pipeline
                for sub in (0, 1):
                    while pend_z[sub] or pend_c[sub]:
                        if pend_z[sub]:
                            zkc, ze_t, zzp, zadd = pend_z[sub].pop(0)
                            mn_t = emit_mn(zkc, sub, zzp, zadd)
                            pend_c[sub].append((ze_t, mn_t))
                        ce_t, cmn_t = pend_c[sub].pop(0)
                        emit_out(ce_t, cmn_t)

            # ---- epilogue: half-bias + store partial (pair-reduced by a
            # grouped psum fused into the same jitted dispatch) --------------
            out_sb = const.tile([128, 1024], F32, tag="out_sb")
            nc.vector.tensor_scalar_add(out_sb[:], out_ps[:], bo2_t[:])
            nc.gpsimd.dma_start(out=out_ext[:, 0:1024], in_=out_sb[0:D, :])
            nc.gpsimd.dma_start(
                out=out_ext[:, 1024:2048], in_=out_sb[D : 2 * D, :]
            )

    _fix_drain_waits(nc)
    return nc


def shard_inputs(q, Wq, bq, Wk, bk, Wv, bv, Wo, bo):
    in_maps = []
    for c in range(N_CORES):
        b, j = c // 2, c % 2
        hs = slice(j * HB, (j + 1) * HB)
        # fold Wo into the V projection per head: W2_h = Wv_h @ Wo_h,
        # b2_h = bv_h @ Wo_h  (out = sum_h attn_h @ vp_h @ Wo_h + bo)
        Wv_c = np.asarray(Wv[:, hs], dtype=np.float64)
        bv_c = np.asarray(bv[hs], dtype=np.float64)
        Wo_c = np.asarray(Wo[hs, :], dtype=np.float64)
        W2 = np.empty((D, HB), dtype=np.float64)
        b2 = np.empty((HB,), dtype=np.float64)
        for l in range(HPC):
            blk = slice(l * D, (l + 1) * D)
            W2[:, blk] = Wv_c[:, blk] @ Wo_c[blk, :]
            b2[blk] = bv_c[blk] @ Wo_c[blk, :]
        in_maps.append(
            {
                "qt": np.ascontiguousarray(q[b].T, dtype=np.float32),
                "wq": np.ascontiguousarray(Wq[:, hs], dtype=np.float32),
                "bq": np.ascontiguousarray(bq[hs], dtype=np.float32),
                "wk": np.ascontiguousarray(Wk[:, hs], dtype=np.float32),
                "bk": np.ascontiguousarray(bk[hs], dtype=np.float32),
                "wv": W2.astype(np.float32),
                "bv": b2.astype(np.float32),
                "bo": np.ascontiguousarray(bo, dtype=np.float32),
            }
        )
    return in_maps


_CACHE = {}


def get_nc():
    if "nc" not in _CACHE:
        _CACHE["nc"] = _build()
    return _CACHE["nc"]


def run_spmd(nc, in_maps):
    """run_bass_via_pjrt with a grouped psum dispatched on-device right
    after the bass NEFF (the NEFF-embedded collective_compute hangs under
    this runtime, so the pair-reduction runs as an XLA collective; the
    bass_exec jit must contain only the custom call, so the psum is its
    own dispatch on device-resident outputs)."""
    import jax
    from jax.sharding import Mesh, PartitionSpec
    from jax.experimental.shard_map import shard_map
    from concourse import bass2jax

    bass2jax.install_neuronx_cc_hook()

    partition_name = nc.partition_id_tensor.name if nc.partition_id_tensor else None
    in_names, out_names, out_avals, zero_outs = [], [], [], []
    for alloc in nc.m.functions[0].allocations:
        if not isinstance(alloc, mybir.MemoryLocationSet):
            continue
        name = alloc.memorylocations[0].name
        if alloc.kind == "ExternalInput":
            if name != partition_name:
                in_names.append(name)
        elif alloc.kind == "ExternalOutput":
            out_names.append(name)
            shape = tuple(alloc.tensor_shape)
            dtype = mybir.dt.np(alloc.dtype)
            out_avals.append(jax.core.ShapedArray(shape, dtype))
            zero_outs.append(np.zeros(shape, dtype))
    n_params = len(in_names)
    n_outs = len(out_avals)
    in_names = in_names + out_names
    if partition_name is not None:
        in_names.append(partition_name)
    donate = tuple(range(n_params, n_params + n_outs))

    def _body(*args):
        operands = list(args)
        if partition_name is not None:
            operands.append(bass2jax.partition_id_tensor())
        outs = bass2jax._bass_exec_p.bind(
            *operands,
            out_avals=tuple(out_avals),
            in_names=tuple(in_names),
            out_names=tuple(out_names),
            lowering_input_output_aliases=(),
            sim_require_finite=True,
            sim_require_nnan=True,
            nc=nc,
        )
        return tuple(outs)

    devices = jax.devices()[:N_CORES]
    mesh = Mesh(np.asarray(devices), ("core",))
    sharded = jax.jit(
        shard_map(
            _body,
            mesh=mesh,
            in_specs=(PartitionSpec("core"),) * (n_params + n_outs),
            out_specs=(PartitionSpec("core"),) * n_outs,
            check_rep=False,
        ),
        donate_argnums=donate,
        keep_unused=True,
    )
    per_core = [[np.asarray(m[name]) for name in in_names[:n_params]] for m in in_maps]
    concat_in = [
        np.concatenate([per_core[c][i] for c in range(N_CORES)], axis=0)
        for i in range(n_params)
    ]
    concat_zeros = [
        np.zeros((N_CORES * z.shape[0], *z.shape[1:]), z.dtype) for z in zero_outs
    ]
    out_arrs = sharded(*concat_in, *concat_zeros)

    # pair-reduce on device: separate dispatch (the bass_exec jit must
    # contain only the custom call, per neuronx_cc_hook's checks)
    def _reduce(*outs):
        return tuple(
            jax.lax.psum(o, "core", axis_index_groups=REPLICA_GROUPS) for o in outs
        )

    reducer = jax.jit(
        shard_map(
            _reduce,
            mesh=mesh,
            in_specs=(PartitionSpec("core"),) * n_outs,
            out_specs=(PartitionSpec("core"),) * n_outs,
            check_rep=False,
        )
    )
    out_arrs = reducer(*out_arrs)
    return [
        {
            name: np.asarray(out_arrs[i]).reshape(N_CORES, *out_avals[i].shape)[c]
            for i, name in enumerate(out_names)
        }
        for c in range(N_CORES)
    ]


def kernel(q, Wq, bq, Wk, bk, Wv, bv, Wo, bo):
    nc = get_nc()
    in_maps = shard_inputs(q, Wq, bq, Wk, bk, Wv, bv, Wo, bo)
    results = run_spmd(nc, in_maps)
    out = np.stack([results[2 * b]["out"].T for b in range(B)], axis=0)
    return out.astype(np.float32)
